# revision 9
# baseline (speedup 1.0000x reference)
"""GAT layer kernel for 8 Trainium2 NeuronCores.

Row-shards the N=8192 destination nodes across 8 cores (1024 rows each).
Per core, for each [128, 8192] row-tile:
  adj  --dma+cast-->  adjf16 (0.0/1.0)
  xf   = g_j + f_i                      (DVE tensor_scalar, per-partition f)
  e    = max(xf, 0.2*xf)                (DVE fused scalar_tensor_tensor: leaky relu)
  z    = exp(e - 2)                     (ACT; the -2 shift cancels in softmax,
                                         keeps z in fp16 range)
  z    = z * adjf16                     (DVE mask; exact zeros for non-edges)
  zT   = blocked transpose of z         (XBAR dma transpose, fp16)
  psum[65,128] = sum_jb haug[jb].T @ zT[:,jb,:]   (PE; haug = [h | ones] so
                                         row 64 is the softmax denominator)
  epilogue: PE-transpose psum, divide by denominator, leaky_relu(0.01), DMA out.

h = input@W, f = h@a[:64], g = h@a[64:] are precomputed on host (the natural
"replicate h" strategy - they are O(N*F) vs the O(N^2) on-device work).
"""

import sys

if "/opt/trn_rl_repo" not in sys.path:
    sys.path.insert(0, "/opt/trn_rl_repo")

import numpy as np

N = 8192
F_OUT = 64
NCORES = 8
ROWS = N // NCORES  # 1024 rows per core
P = 128
NT = ROWS // P      # 8 row-tiles per core
JB = N // P         # 64 column blocks
HA_W = F_OUT + 1    # h features + ones column
CS = 2.0            # exp shift (cancels in softmax)

_nc_cache = [None]


def build_bass():
    from contextlib import ExitStack

    import concourse.bass as bass
    import concourse.bacc as bacc
    import concourse.tile as tile
    from concourse import mybir
    from concourse.masks import make_identity

    f16 = mybir.dt.float16
    f32 = mybir.dt.float32
    i32 = mybir.dt.int32
    Alu = mybir.AluOpType
    Act = mybir.ActivationFunctionType

    nc = bacc.Bacc()
    adj_d = nc.declare_dram_parameter("adjs", [ROWS, N], i32, isOutput=False)
    g_d = nc.declare_dram_parameter("gbc", [P, N], f16, isOutput=False)
    ha_d = nc.declare_dram_parameter("haug", [P, JB, HA_W], f16, isOutput=False)
    f_d = nc.declare_dram_parameter("fsc", [P, NT], f32, isOutput=False)
    out_d = nc.declare_dram_parameter("out", [ROWS, F_OUT], f32, isOutput=True)

    with ExitStack() as ctx:
        tc = ctx.enter_context(tile.TileContext(nc))
        singles = ctx.enter_context(tc.tile_pool(name="singles", bufs=1))
        adjp = ctx.enter_context(tc.tile_pool(name="adjp", bufs=2))
        xp = ctx.enter_context(tc.tile_pool(name="xp", bufs=2))
        zp = ctx.enter_context(tc.tile_pool(name="zp", bufs=2))
        ztp = ctx.enter_context(tc.tile_pool(name="ztp", bufs=2))
        smalls = ctx.enter_context(tc.tile_pool(name="smalls", bufs=3))
        psp = ctx.enter_context(tc.tile_pool(name="psp", bufs=2, space="PSUM"))
        pst = ctx.enter_context(tc.tile_pool(name="pst", bufs=2, space="PSUM"))

        G = singles.tile([P, N], f16)
        nc.sync.dma_start(out=G, in_=g_d[:, :])
        HA = singles.tile([P, JB, HA_W], f16)
        nc.sync.dma_start(out=HA, in_=ha_d[:, :, :])
        FS = singles.tile([P, NT], f32)
        nc.sync.dma_start(out=FS, in_=f_d[:, :])
        # FS2 = 0.2 * FS, for the leaky-relu negative branch
        FS2 = singles.tile([P, NT], f32)
        nc.vector.tensor_scalar(
            out=FS2, in0=FS, scalar1=0.2, scalar2=None, op0=Alu.mult
        )
        IDT = singles.tile([P, P], f32)
        make_identity(nc, IDT)
        NEGCS = singles.tile([P, 1], f32)
        nc.vector.memset(NEGCS, -CS)

        for t in range(NT):
            # adjacency row-tile, cast int32 -> fp16 during the DMA (SWDGE)
            adjf = adjp.tile([P, N], f16, tag="adjf")
            nc.gpsimd.dma_start(out=adjf, in_=adj_d[t * P : (t + 1) * P, :])

            # xf = g_j + f_i ; xf2 = 0.2*(g_j + f_i) ; e = max(xf, xf2) = leaky_relu
            xf = xp.tile([P, N], f16, tag="xf")
            nc.vector.tensor_scalar(
                out=xf, in0=G, scalar1=FS[:, t : t + 1], scalar2=None, op0=Alu.add
            )
            xf2 = xp.tile([P, N], f16, tag="xf2")
            nc.vector.tensor_scalar(
                out=xf2,
                in0=G,
                scalar1=0.2,
                scalar2=FS2[:, t : t + 1],
                op0=Alu.mult,
                op1=Alu.add,
            )
            z = zp.tile([P, N], f16, tag="z")
            nc.vector.tensor_tensor(out=z, in0=xf, in1=xf2, op=Alu.max)
            # z = exp(e - CS)
            nc.scalar.activation(out=z, in_=z, func=Act.Exp, bias=NEGCS[:, :], scale=1.0)
            # mask: z *= adj
            nc.vector.tensor_tensor(out=z, in0=z, in1=adjf, op=Alu.mult)

            # blocked transpose: zt[jj, b, ii] = z[ii, b*128+jj]
            zt = ztp.tile([P, JB, P], f16, tag="zt")
            nc.sync.dma_start(out=zt, in_=z, transpose=True)

            # psum[m, i] = sum_j haug[j, m] * z[i, j]  (m=0..63 features, m=64 denom)
            ps = psp.tile([HA_W, P], f32, tag="ps")
            for jb in range(JB):
                nc.tensor.matmul(
                    ps,
                    HA[:, jb, :],
                    zt[:, jb, :],
                    start=(jb == 0),
                    stop=(jb == JB - 1),
                )

            # epilogue: transpose [65,128] -> [128,65], normalize, leaky_relu(0.01)
            sb1 = smalls.tile([HA_W, P], f32, tag="sb1")
            nc.vector.tensor_copy(out=sb1, in_=ps)
            ps2 = pst.tile([P, HA_W], f32, tag="ps2")
            nc.tensor.transpose(ps2, sb1, IDT[:HA_W, :HA_W])
            sb2 = smalls.tile([P, HA_W], f32, tag="sb2")
            nc.vector.tensor_copy(out=sb2, in_=ps2)
            rec = smalls.tile([P, 1], f32, tag="rec")
            nc.vector.reciprocal(rec, sb2[:, F_OUT : F_OUT + 1])
            res = smalls.tile([P, F_OUT], f32, tag="res")
            nc.vector.tensor_scalar(
                out=res, in0=sb2[:, 0:F_OUT], scalar1=rec, scalar2=None, op0=Alu.mult
            )
            res2 = smalls.tile([P, F_OUT], f32, tag="res2")
            nc.vector.tensor_scalar(
                out=res2, in0=res, scalar1=0.01, scalar2=None, op0=Alu.mult
            )
            fin = smalls.tile([P, F_OUT], f32, tag="fin")
            nc.vector.tensor_tensor(out=fin, in0=res, in1=res2, op=Alu.max)
            nc.sync.dma_start(out=out_d[t * P : (t + 1) * P, :], in_=fin)
    nc.finalize()
    return nc


def prep_inputs(input, adj, W, a):
    """Host-side prep: h = input@W, f/g attention projections, fp16 packing."""
    h = np.asarray(input, np.float32) @ np.asarray(W, np.float32)
    av = np.asarray(a, np.float32).reshape(2 * F_OUT)
    f = (h @ av[:F_OUT]).astype(np.float32)
    g = (h @ av[F_OUT:]).astype(np.float32)
    gbc = np.ascontiguousarray(np.broadcast_to(g.astype(np.float16), (P, N)))
    ha = np.concatenate([h, np.ones((N, 1), np.float32)], axis=1).astype(np.float16)
    ha_r = np.ascontiguousarray(ha.reshape(JB, P, HA_W).transpose(1, 0, 2))
    adj = np.asarray(adj, np.int32)
    in_maps = []
    for c in range(NCORES):
        fc = f[c * ROWS : (c + 1) * ROWS]
        in_maps.append(
            {
                "adjs": np.ascontiguousarray(adj[c * ROWS : (c + 1) * ROWS]),
                "gbc": gbc,
                "haug": ha_r,
                "fsc": np.ascontiguousarray(fc.reshape(NT, P).T),
            }
        )
    return in_maps


def kernel(input, adj, W, a, sparse):
    from concourse.bass_utils import run_bass_kernel_spmd

    in_maps = prep_inputs(input, adj, W, a)
    if _nc_cache[0] is None:
        _nc_cache[0] = build_bass()
    nc = _nc_cache[0]
    r = run_bass_kernel_spmd(nc, in_maps, list(range(NCORES)))
    out = np.concatenate([np.asarray(r.results[c]["out"]) for c in range(NCORES)], axis=0)
    return out[None].astype(np.float32)


# revision 16
# speedup vs baseline: 1.9206x; 1.9206x over previous
"""GAT layer kernel for 8 Trainium2 NeuronCores.

Row-shards the N=8192 destination nodes across 8 cores (1024 rows each).
Each core receives its adjacency slice pre-transposed (adjT[j, i] =
adj[row_i, j], same bytes, sharding layout choice), so all compute runs in
the matmul-ready [j, i] layout and no on-device transpose is needed.

Per core, 8 "stacks", each stack s covering 8 j-blocks of 128 (j in
[s*1024, (s+1)*1024)), as one [128, 8*1024] fp16 tile:
  adjT --dma+cast--> adjf16 (0.0/1.0)                       (SWDGE, int32->fp16)
  per j-block b: z[:,b,:] = leaky_relu(f_i + g_j, 0.2)
      ACT path:  Prelu(in=F, bias=g_b, alpha=0.2)           (1 op/block)
      DVE path:  ts add (f + g_b), then stacked in-place
                 scalar_tensor_tensor max(0.2*z, z)
  z = exp(z - 2)           stacked, ACT (the -2 cancels in softmax; keeps fp16)
  z = z * adjf16           stacked, DVE (mask -> exact zeros)
  per block b, i-chunk c:  psum_c[65, 512] += haug[b].T @ z  (PE; haug=[h|1],
                           row 64 accumulates the softmax denominator)
Epilogue: PE-transpose psum 128-col slices, divide rows by the denominator,
Prelu(0.01), DMA out (natural [1024, 64] layout).

h = input@W, f = h@a[:64], g = h@a[64:] are precomputed on host (the
"replicate h" strategy from the sharding hint - O(N*F) vs O(N^2) on device).
"""

import sys

if "/opt/trn_rl_repo" not in sys.path:
    sys.path.insert(0, "/opt/trn_rl_repo")

import numpy as np

N = 8192
F_OUT = 64
NCORES = 8
ROWS = N // NCORES  # 1024 rows per core
P = 128
JB = N // P         # 64 j-blocks
NS = 8              # stacks per core
KB = JB // NS       # 8 j-blocks per stack
HA_W = F_OUT + 1    # h features + ones column
CS = 2.0            # exp shift (cancels in softmax)
ACT_STACKS = (1, 3, 6)  # stacks whose leaky-relu runs on ACT (Prelu)

_nc_cache = {}


def build_bass(act_stacks=ACT_STACKS):
    from contextlib import ExitStack

    import concourse.bacc as bacc
    import concourse.tile as tile
    from concourse import mybir
    from concourse.masks import make_identity

    f16 = mybir.dt.float16
    f32 = mybir.dt.float32
    i32 = mybir.dt.int32
    Alu = mybir.AluOpType
    Act = mybir.ActivationFunctionType

    nc = bacc.Bacc()
    adj_d = nc.declare_dram_parameter("adjt", [N, ROWS], i32, isOutput=False)
    fb_d = nc.declare_dram_parameter("fbig", [P, ROWS], f16, isOutput=False)
    g_d = nc.declare_dram_parameter("gsc", [P, JB], f32, isOutput=False)
    ha_d = nc.declare_dram_parameter("haug", [P, JB, HA_W], f16, isOutput=False)
    out_d = nc.declare_dram_parameter("out", [ROWS, F_OUT], f32, isOutput=True)

    with ExitStack() as ctx:
        tc = ctx.enter_context(tile.TileContext(nc))
        singles = ctx.enter_context(tc.tile_pool(name="singles", bufs=1))
        adjp = ctx.enter_context(tc.tile_pool(name="adjp", bufs=3))
        zp = ctx.enter_context(tc.tile_pool(name="zp", bufs=3))
        smalls = ctx.enter_context(tc.tile_pool(name="smalls", bufs=3))
        psp = ctx.enter_context(tc.tile_pool(name="psp", bufs=1, space="PSUM"))
        pst = ctx.enter_context(tc.tile_pool(name="pst", bufs=2, space="PSUM"))

        FB = singles.tile([P, ROWS], f16)
        nc.sync.dma_start(out=FB, in_=fb_d[:, :])
        GS = singles.tile([P, JB], f32)
        nc.sync.dma_start(out=GS, in_=g_d[:, :])
        HA = singles.tile([P, JB, HA_W], f16)
        nc.sync.dma_start(out=HA, in_=ha_d[:, :, :])
        IDT = singles.tile([P, P], f32)
        make_identity(nc, IDT)
        NEGCS = singles.tile([P, 1], f32)
        nc.vector.memset(NEGCS, -CS)

        # persistent psum accumulators, one per 512-wide i-chunk
        ps = [
            psp.tile([HA_W, 512], f32, tag=f"ps{c}", name=f"ps{c}") for c in range(2)
        ]

        for s in range(NS):
            adjf = adjp.tile([P, KB, ROWS], f16, tag="adjf")
            src = adj_d[s * KB * P : (s + 1) * KB * P, :].rearrange(
                "(k p) i -> p k i", p=P
            )
            nc.gpsimd.dma_start(out=adjf, in_=src)

            z = zp.tile([P, KB, ROWS], f16, tag="z")
            if s in act_stacks:
                for kk in range(KB):
                    b = s * KB + kk
                    nc.scalar.activation(
                        out=z[:, kk, :],
                        in_=FB[:, :],
                        func=Act.Prelu,
                        bias=GS[:, b : b + 1],
                        scale=1.0,
                        alpha=0.2,
                    )
            else:
                for kk in range(KB):
                    b = s * KB + kk
                    nc.vector.tensor_scalar(
                        out=z[:, kk, :],
                        in0=FB[:, :],
                        scalar1=GS[:, b : b + 1],
                        scalar2=None,
                        op0=Alu.add,
                    )
                nc.vector.scalar_tensor_tensor(
                    out=z[:, :, :], in0=z[:, :, :], scalar=0.2, in1=z[:, :, :],
                    op0=Alu.mult, op1=Alu.max,
                )
            nc.scalar.activation(
                out=z[:, :, :], in_=z[:, :, :], func=Act.Exp,
                bias=NEGCS[:, :], scale=1.0,
            )
            nc.vector.tensor_tensor(
                out=z[:, :, :], in0=z[:, :, :], in1=adjf[:, :, :], op=Alu.mult
            )

            for kk in range(KB):
                b = s * KB + kk
                for c in range(2):
                    nc.tensor.matmul(
                        ps[c],
                        HA[:, b, :],
                        z[:, kk, c * 512 : (c + 1) * 512],
                        start=(b == 0),
                        stop=(b == JB - 1),
                    )

        # epilogue: transpose psum slices, normalize, leaky_relu(0.01)
        for c in range(2):
            sb1 = smalls.tile([HA_W, 512], f32, tag="sb1")
            nc.vector.tensor_copy(out=sb1, in_=ps[c])
            for q in range(4):
                t = c * 4 + q
                ps2 = pst.tile([P, HA_W], f32, tag="ps2")
                nc.tensor.transpose(
                    ps2, sb1[:, q * P : (q + 1) * P], IDT[:HA_W, :HA_W]
                )
                sb2 = smalls.tile([P, HA_W], f32, tag="sb2")
                nc.vector.tensor_copy(out=sb2, in_=ps2)
                rec = smalls.tile([P, 1], f32, tag="rec")
                nc.vector.reciprocal(rec, sb2[:, F_OUT : F_OUT + 1])
                res = smalls.tile([P, F_OUT], f32, tag="res")
                nc.vector.tensor_scalar(
                    out=res, in0=sb2[:, 0:F_OUT], scalar1=rec, scalar2=None,
                    op0=Alu.mult,
                )
                fin = smalls.tile([P, F_OUT], f32, tag="fin")
                if act_stacks:
                    nc.scalar.activation(
                        out=fin, in_=res, func=Act.Prelu, bias=0.0, scale=1.0,
                        alpha=0.01,
                    )
                else:  # sim path (interp lacks Prelu)
                    nc.vector.scalar_tensor_tensor(
                        out=fin, in0=res, scalar=0.01, in1=res,
                        op0=Alu.mult, op1=Alu.max,
                    )
                nc.sync.dma_start(out=out_d[t * P : (t + 1) * P, :], in_=fin)
    nc.finalize()
    return nc


def prep_inputs(input, adj, W, a):
    """Host-side prep: h = input@W, f/g projections, per-core sharding layout."""
    h = np.asarray(input, np.float32) @ np.asarray(W, np.float32)
    av = np.asarray(a, np.float32).reshape(2 * F_OUT)
    f = (h @ av[:F_OUT]).astype(np.float32)
    g = (h @ av[F_OUT:]).astype(np.float32)
    gs = np.ascontiguousarray(g.reshape(JB, P).T)  # gs[p, b] = g[b*128+p]
    ha = np.concatenate([h, np.ones((N, 1), np.float32)], axis=1).astype(np.float16)
    ha_r = np.ascontiguousarray(ha.reshape(JB, P, HA_W).transpose(1, 0, 2))
    adj = np.asarray(adj, np.int32)
    in_maps = []
    for c in range(NCORES):
        fc = f[c * ROWS : (c + 1) * ROWS].astype(np.float16)
        in_maps.append(
            {
                "adjt": np.ascontiguousarray(adj[c * ROWS : (c + 1) * ROWS].T),
                "fbig": np.ascontiguousarray(
                    np.broadcast_to(fc[None, :], (P, ROWS))
                ),
                "gsc": gs,
                "haug": ha_r,
            }
        )
    return in_maps


def kernel(input, adj, W, a, sparse):
    from concourse.bass_utils import run_bass_kernel_spmd

    in_maps = prep_inputs(input, adj, W, a)
    if "nc" not in _nc_cache:
        _nc_cache["nc"] = build_bass()
    nc = _nc_cache["nc"]
    r = run_bass_kernel_spmd(nc, in_maps, list(range(NCORES)))
    out = np.concatenate(
        [np.asarray(r.results[c]["out"]) for c in range(NCORES)], axis=0
    )
    return out[None].astype(np.float32)


# revision 19
# speedup vs baseline: 3.2311x; 1.6823x over previous
"""GAT layer kernel for 8 Trainium2 NeuronCores.

Row-shards the N=8192 destination nodes across 8 cores (1024 rows each).
Each core receives its adjacency slice pre-transposed (adjT[j, i] =
adj[row_i, j], same bytes, sharding layout choice), so all compute runs in
the matmul-ready [j, i] layout and no on-device transpose is needed.

Per core, 8 "stacks", each stack s covering 8 j-blocks of 128 (j in
[s*1024, (s+1)*1024)), as one [128, 8*1024] fp16 tile:
  adjT --dma+cast--> adjf16 (0.0/1.0)                       (SWDGE, int32->fp16)
  per j-block b: z[:,b,:] = leaky_relu(f_i + g_j, 0.2)
      ACT path:  Prelu(in=F, bias=g_b, alpha=0.2)           (1 op/block)
      DVE path:  ts add (f + g_b), then stacked in-place
                 scalar_tensor_tensor max(0.2*z, z)
  z = exp(z - 2)           stacked, ACT (the -2 cancels in softmax; keeps fp16)
  z = z * adjf16           stacked, DVE (mask -> exact zeros)
  per block b, i-chunk c:  psum_c[65, 512] += haug[b].T @ z  (PE; haug=[h|1],
                           row 64 accumulates the softmax denominator)
Epilogue: PE-transpose psum 128-col slices, divide rows by the denominator,
Prelu(0.01), DMA out (natural [1024, 64] layout).

h = input@W, f = h@a[:64], g = h@a[64:] are precomputed on host (the
"replicate h" strategy from the sharding hint - O(N*F) vs O(N^2) on device).
"""

import sys

if "/opt/trn_rl_repo" not in sys.path:
    sys.path.insert(0, "/opt/trn_rl_repo")

import numpy as np

N = 8192
F_OUT = 64
NCORES = 8
ROWS = N // NCORES  # 1024 rows per core
P = 128
JB = N // P         # 64 j-blocks
NS = 8              # stacks per core
KB = JB // NS       # 8 j-blocks per stack
HA_W = F_OUT + 1    # h features + ones column
CS = 2.0            # exp shift (cancels in softmax)
ACT_STACKS = (1, 3, 6)  # stacks whose leaky-relu runs on ACT (Prelu)

_nc_cache = {}


def build_bass(act_stacks=ACT_STACKS, reps=1):
    from contextlib import ExitStack

    import concourse.bacc as bacc
    import concourse.tile as tile
    from concourse import mybir
    from concourse.masks import make_identity

    f16 = mybir.dt.float16
    f32 = mybir.dt.float32
    i32 = mybir.dt.int32
    Alu = mybir.AluOpType
    Act = mybir.ActivationFunctionType

    nc = bacc.Bacc()
    adj_d = nc.declare_dram_parameter("adjt", [N, ROWS], i32, isOutput=False)
    fb_d = nc.declare_dram_parameter("fbig", [P, ROWS], f16, isOutput=False)
    g_d = nc.declare_dram_parameter("gsc", [P, JB], f32, isOutput=False)
    ha_d = nc.declare_dram_parameter("haug", [P, JB, HA_W], f16, isOutput=False)
    out_d = nc.declare_dram_parameter("out", [ROWS, F_OUT], f32, isOutput=True)

    with ExitStack() as ctx:
        tc = ctx.enter_context(tile.TileContext(nc))
        singles = ctx.enter_context(tc.tile_pool(name="singles", bufs=1))
        adjp = ctx.enter_context(tc.tile_pool(name="adjp", bufs=3))
        zp = ctx.enter_context(tc.tile_pool(name="zp", bufs=3))
        smalls = ctx.enter_context(tc.tile_pool(name="smalls", bufs=3))
        psp = ctx.enter_context(tc.tile_pool(name="psp", bufs=1, space="PSUM"))
        pst = ctx.enter_context(tc.tile_pool(name="pst", bufs=2, space="PSUM"))

        FB = singles.tile([P, ROWS], f16)
        nc.sync.dma_start(out=FB, in_=fb_d[:, :])
        GS = singles.tile([P, JB], f32)
        nc.sync.dma_start(out=GS, in_=g_d[:, :])
        HA = singles.tile([P, JB, HA_W], f16)
        nc.sync.dma_start(out=HA, in_=ha_d[:, :, :])
        IDT = singles.tile([P, P], f32)
        make_identity(nc, IDT)
        NEGCS = singles.tile([P, 1], f32)
        nc.vector.memset(NEGCS, -CS)

        # persistent psum accumulators, one per 512-wide i-chunk
        ps = [
            psp.tile([HA_W, 512], f32, tag=f"ps{c}", name=f"ps{c}") for c in range(2)
        ]

        rep_ctx = ExitStack()
        if reps > 1:  # timing mode: repeat the whole body inside the NEFF
            rep_ctx.enter_context(tc.For_i(0, reps, 1))

        for s in range(NS):
            adjf = adjp.tile([P, KB, ROWS], f16, tag="adjf")
            src = adj_d[s * KB * P : (s + 1) * KB * P, :].rearrange(
                "(k p) i -> p k i", p=P
            )
            nc.gpsimd.dma_start(out=adjf, in_=src)

            z = zp.tile([P, KB, ROWS], f16, tag="z")
            if s in act_stacks:
                for kk in range(KB):
                    b = s * KB + kk
                    nc.scalar.activation(
                        out=z[:, kk, :],
                        in_=FB[:, :],
                        func=Act.Prelu,
                        bias=GS[:, b : b + 1],
                        scale=1.0,
                        alpha=0.2,
                    )
            else:
                for kk in range(KB):
                    b = s * KB + kk
                    nc.vector.tensor_scalar(
                        out=z[:, kk, :],
                        in0=FB[:, :],
                        scalar1=GS[:, b : b + 1],
                        scalar2=None,
                        op0=Alu.add,
                    )
                nc.vector.scalar_tensor_tensor(
                    out=z[:, :, :], in0=z[:, :, :], scalar=0.2, in1=z[:, :, :],
                    op0=Alu.mult, op1=Alu.max,
                )
            nc.scalar.activation(
                out=z[:, :, :], in_=z[:, :, :], func=Act.Exp,
                bias=NEGCS[:, :], scale=1.0,
            )
            nc.vector.tensor_tensor(
                out=z[:, :, :], in0=z[:, :, :], in1=adjf[:, :, :], op=Alu.mult
            )

            for kk in range(KB):
                b = s * KB + kk
                for c in range(2):
                    nc.tensor.matmul(
                        ps[c],
                        HA[:, b, :],
                        z[:, kk, c * 512 : (c + 1) * 512],
                        start=(b == 0),
                        stop=(b == JB - 1),
                    )

        # epilogue: transpose psum slices, normalize, leaky_relu(0.01)
        for c in range(2):
            sb1 = smalls.tile([HA_W, 512], f32, tag="sb1")
            nc.vector.tensor_copy(out=sb1, in_=ps[c])
            for q in range(4):
                t = c * 4 + q
                ps2 = pst.tile([P, HA_W], f32, tag="ps2")
                nc.tensor.transpose(
                    ps2, sb1[:, q * P : (q + 1) * P], IDT[:HA_W, :HA_W]
                )
                sb2 = smalls.tile([P, HA_W], f32, tag="sb2")
                nc.vector.tensor_copy(out=sb2, in_=ps2)
                rec = smalls.tile([P, 1], f32, tag="rec")
                nc.vector.reciprocal(rec, sb2[:, F_OUT : F_OUT + 1])
                res = smalls.tile([P, F_OUT], f32, tag="res")
                nc.vector.tensor_scalar(
                    out=res, in0=sb2[:, 0:F_OUT], scalar1=rec, scalar2=None,
                    op0=Alu.mult,
                )
                fin = smalls.tile([P, F_OUT], f32, tag="fin")
                if act_stacks:
                    nc.scalar.activation(
                        out=fin, in_=res, func=Act.Prelu, bias=0.0, scale=1.0,
                        alpha=0.01,
                    )
                else:  # sim path (interp lacks Prelu)
                    nc.vector.scalar_tensor_tensor(
                        out=fin, in0=res, scalar=0.01, in1=res,
                        op0=Alu.mult, op1=Alu.max,
                    )
                nc.sync.dma_start(out=out_d[t * P : (t + 1) * P, :], in_=fin)
        rep_ctx.close()
    nc.finalize()
    return nc


def prep_inputs(input, adj, W, a):
    """Host-side prep: h = input@W, f/g projections, per-core sharding layout."""
    h = np.asarray(input, np.float32) @ np.asarray(W, np.float32)
    av = np.asarray(a, np.float32).reshape(2 * F_OUT)
    f = (h @ av[:F_OUT]).astype(np.float32)
    g = (h @ av[F_OUT:]).astype(np.float32)
    gs = np.ascontiguousarray(g.reshape(JB, P).T)  # gs[p, b] = g[b*128+p]
    ha = np.concatenate([h, np.ones((N, 1), np.float32)], axis=1).astype(np.float16)
    ha_r = np.ascontiguousarray(ha.reshape(JB, P, HA_W).transpose(1, 0, 2))
    adj = np.asarray(adj, np.int32)
    in_maps = []
    for c in range(NCORES):
        fc = f[c * ROWS : (c + 1) * ROWS].astype(np.float16)
        in_maps.append(
            {
                "adjt": np.ascontiguousarray(adj[c * ROWS : (c + 1) * ROWS].T),
                "fbig": np.ascontiguousarray(
                    np.broadcast_to(fc[None, :], (P, ROWS))
                ),
                "gsc": gs,
                "haug": ha_r,
            }
        )
    return in_maps


def kernel(input, adj, W, a, sparse):
    from concourse.bass_utils import run_bass_kernel_spmd

    in_maps = prep_inputs(input, adj, W, a)
    if "nc" not in _nc_cache:
        _nc_cache["nc"] = build_bass()
    nc = _nc_cache["nc"]
    r = run_bass_kernel_spmd(nc, in_maps, list(range(NCORES)))
    out = np.concatenate(
        [np.asarray(r.results[c]["out"]) for c in range(NCORES)], axis=0
    )
    return out[None].astype(np.float32)


# revision 23
# speedup vs baseline: 3.6658x; 1.1346x over previous
"""GAT layer kernel for 8 Trainium2 NeuronCores.

Row-shards the N=8192 destination nodes across 8 cores (1024 rows each).
Each core receives its adjacency slice pre-transposed (adjT[j, i] =
adj[row_i, j], same bytes, sharding layout choice), so all compute runs in
the matmul-ready [j, i] layout and no on-device transpose is needed.

Per core, 8 "stacks", each stack s covering 8 j-blocks of 128 (j in
[s*1024, (s+1)*1024)), as one [128, 8*1024] fp16 tile:
  adjT --dma+cast--> adjf16 (0.0/1.0)                       (SWDGE, int32->fp16)
  per j-block b: z[:,b,:] = leaky_relu(f_i + g_j, 0.2)
      ACT path:  Prelu(in=F, bias=g_b, alpha=0.2)           (1 op/block)
      DVE path:  ts add (f + g_b), then stacked in-place
                 scalar_tensor_tensor max(0.2*z, z)
  z = exp(z - 2)           stacked, ACT (the -2 cancels in softmax; keeps fp16)
  z = z * adjf16           stacked, DVE (mask -> exact zeros)
  per block b, i-chunk c:  psum_c[65, 512] += haug[b].T @ z  (PE; haug=[h|1],
                           row 64 accumulates the softmax denominator)
Epilogue: PE-transpose psum 128-col slices, divide rows by the denominator,
Prelu(0.01), DMA out (natural [1024, 64] layout).

h = input@W, f = h@a[:64], g = h@a[64:] are precomputed on host (the
"replicate h" strategy from the sharding hint - O(N*F) vs O(N^2) on device).
"""

import sys

if "/opt/trn_rl_repo" not in sys.path:
    sys.path.insert(0, "/opt/trn_rl_repo")

import numpy as np

N = 8192
F_OUT = 64
NCORES = 8
ROWS = N // NCORES  # 1024 rows per core
P = 128
JB = N // P         # 64 j-blocks
NS = 8              # stacks per core
KB = JB // NS       # 8 j-blocks per stack
HA_W = F_OUT + 1    # h features + ones column
CS = 2.0            # exp shift (cancels in softmax)
ACT_STACKS = (1, 3, 6)  # stacks whose leaky-relu runs on ACT (Prelu)

_nc_cache = {}


def build_bass(act_stacks=ACT_STACKS, reps=1, sim_relu=False):
    from contextlib import ExitStack

    import concourse.bacc as bacc
    import concourse.tile as tile
    from concourse import mybir
    from concourse.masks import make_identity

    f16 = mybir.dt.float16
    f32 = mybir.dt.float32
    i32 = mybir.dt.int32
    Alu = mybir.AluOpType
    Act = mybir.ActivationFunctionType

    # sim_relu: the interp lacks Prelu; Relu has identical cost (timing-only runs)
    PRELU = Act.Relu if sim_relu else Act.Prelu

    nc = bacc.Bacc()
    adj_d = nc.declare_dram_parameter("adjt", [N, ROWS], i32, isOutput=False)
    fb_d = nc.declare_dram_parameter("fbig", [P, ROWS], f16, isOutput=False)
    g_d = nc.declare_dram_parameter("gsc", [P, JB], f32, isOutput=False)
    ha_d = nc.declare_dram_parameter("haug", [P, JB, HA_W], f16, isOutput=False)
    out_d = nc.declare_dram_parameter("out", [ROWS, F_OUT], f32, isOutput=True)

    with ExitStack() as ctx:
        tc = ctx.enter_context(tile.TileContext(nc))
        singles = ctx.enter_context(tc.tile_pool(name="singles", bufs=1))
        adjp = ctx.enter_context(tc.tile_pool(name="adjp", bufs=4))
        zp = ctx.enter_context(tc.tile_pool(name="zp", bufs=4))
        smalls = ctx.enter_context(tc.tile_pool(name="smalls", bufs=3))
        psp = ctx.enter_context(tc.tile_pool(name="psp", bufs=1, space="PSUM"))
        pst = ctx.enter_context(tc.tile_pool(name="pst", bufs=2, space="PSUM"))

        FB = singles.tile([P, ROWS], f16)
        nc.sync.dma_start(out=FB, in_=fb_d[:, :])
        GS = singles.tile([P, JB], f32)
        nc.sync.dma_start(out=GS, in_=g_d[:, :])
        HA = singles.tile([P, JB, HA_W], f16)
        nc.sync.dma_start(out=HA, in_=ha_d[:, :, :])
        IDT = singles.tile([P, P], f32)
        make_identity(nc, IDT)
        NEGCS = singles.tile([P, 1], f32)
        nc.vector.memset(NEGCS, -CS)

        # persistent psum accumulators, one per 512-wide i-chunk
        ps = [
            psp.tile([HA_W, 512], f32, tag=f"ps{c}", name=f"ps{c}") for c in range(2)
        ]

        rep_ctx = ExitStack()
        if reps > 1:  # timing mode: repeat the whole body inside the NEFF
            rep_ctx.enter_context(tc.For_i(0, reps, 1))

        for s in range(NS):
            adjf = adjp.tile([P, KB, ROWS], f16, tag="adjf")
            src = adj_d[s * KB * P : (s + 1) * KB * P, :].rearrange(
                "(k p) i -> p k i", p=P
            )
            nc.gpsimd.dma_start(out=adjf, in_=src)

            z = zp.tile([P, KB, ROWS], f16, tag="z")
            if s in act_stacks:
                for kk in range(KB):
                    b = s * KB + kk
                    nc.scalar.activation(
                        out=z[:, kk, :],
                        in_=FB[:, :],
                        func=PRELU,
                        bias=GS[:, b : b + 1],
                        scale=1.0,
                        alpha=0.2,
                    )
            else:
                for kk in range(KB):
                    b = s * KB + kk
                    nc.vector.tensor_scalar(
                        out=z[:, kk, :],
                        in0=FB[:, :],
                        scalar1=GS[:, b : b + 1],
                        scalar2=None,
                        op0=Alu.add,
                    )
                nc.vector.scalar_tensor_tensor(
                    out=z[:, :, :], in0=z[:, :, :], scalar=0.2, in1=z[:, :, :],
                    op0=Alu.mult, op1=Alu.max,
                )
            nc.scalar.activation(
                out=z[:, :, :], in_=z[:, :, :], func=Act.Exp,
                bias=NEGCS[:, :], scale=1.0,
            )
            nc.vector.tensor_tensor(
                out=z[:, :, :], in0=z[:, :, :], in1=adjf[:, :, :], op=Alu.mult
            )

            for kk in range(KB):
                b = s * KB + kk
                for c in range(2):
                    nc.tensor.matmul(
                        ps[c],
                        HA[:, b, :],
                        z[:, kk, c * 512 : (c + 1) * 512],
                        start=(b == 0),
                        stop=(b == JB - 1),
                    )

        # epilogue: transpose psum slices, normalize, leaky_relu(0.01)
        for c in range(2):
            sb1 = smalls.tile([HA_W, 512], f32, tag="sb1")
            nc.vector.tensor_copy(out=sb1, in_=ps[c])
            for q in range(4):
                t = c * 4 + q
                ps2 = pst.tile([P, HA_W], f32, tag="ps2")
                nc.tensor.transpose(
                    ps2, sb1[:, q * P : (q + 1) * P], IDT[:HA_W, :HA_W]
                )
                sb2 = smalls.tile([P, HA_W], f32, tag="sb2")
                nc.vector.tensor_copy(out=sb2, in_=ps2)
                rec = smalls.tile([P, 1], f32, tag="rec")
                nc.vector.reciprocal(rec, sb2[:, F_OUT : F_OUT + 1])
                res = smalls.tile([P, F_OUT], f32, tag="res")
                nc.vector.tensor_scalar(
                    out=res, in0=sb2[:, 0:F_OUT], scalar1=rec, scalar2=None,
                    op0=Alu.mult,
                )
                fin = smalls.tile([P, F_OUT], f32, tag="fin")
                if act_stacks:
                    nc.scalar.activation(
                        out=fin, in_=res, func=PRELU, bias=0.0, scale=1.0,
                        alpha=0.01,
                    )
                else:  # sim path (interp lacks Prelu)
                    nc.vector.scalar_tensor_tensor(
                        out=fin, in0=res, scalar=0.01, in1=res,
                        op0=Alu.mult, op1=Alu.max,
                    )
                nc.sync.dma_start(out=out_d[t * P : (t + 1) * P, :], in_=fin)
        rep_ctx.close()
    nc.finalize()
    return nc


def prep_inputs(input, adj, W, a):
    """Host-side prep: h = input@W, f/g projections, per-core sharding layout."""
    h = np.asarray(input, np.float32) @ np.asarray(W, np.float32)
    av = np.asarray(a, np.float32).reshape(2 * F_OUT)
    f = (h @ av[:F_OUT]).astype(np.float32)
    g = (h @ av[F_OUT:]).astype(np.float32)
    gs = np.ascontiguousarray(g.reshape(JB, P).T)  # gs[p, b] = g[b*128+p]
    ha = np.concatenate([h, np.ones((N, 1), np.float32)], axis=1).astype(np.float16)
    ha_r = np.ascontiguousarray(ha.reshape(JB, P, HA_W).transpose(1, 0, 2))
    adj = np.asarray(adj, np.int32)
    in_maps = []
    for c in range(NCORES):
        fc = f[c * ROWS : (c + 1) * ROWS].astype(np.float16)
        in_maps.append(
            {
                "adjt": np.ascontiguousarray(adj[c * ROWS : (c + 1) * ROWS].T),
                "fbig": np.ascontiguousarray(
                    np.broadcast_to(fc[None, :], (P, ROWS))
                ),
                "gsc": gs,
                "haug": ha_r,
            }
        )
    return in_maps


def kernel(input, adj, W, a, sparse):
    from concourse.bass_utils import run_bass_kernel_spmd

    in_maps = prep_inputs(input, adj, W, a)
    if "nc" not in _nc_cache:
        _nc_cache["nc"] = build_bass()
    nc = _nc_cache["nc"]
    r = run_bass_kernel_spmd(nc, in_maps, list(range(NCORES)))
    out = np.concatenate(
        [np.asarray(r.results[c]["out"]) for c in range(NCORES)], axis=0
    )
    return out[None].astype(np.float32)
